# revision 1
# baseline (speedup 1.0000x reference)
"""Trainium2 Bass kernel for nn_AttentionModulatedOrdinalEmbedding.

Contract: kernel(**inputs) takes the FULL (unsharded) inputs from
setup_inputs() and returns the FULL (B, S, EMB) float32 output.
Internally shards batch-parallel across 8 NeuronCores (4 batches/core),
runs one SPMD Bass kernel, and concatenates the per-core outputs.

Hardcoded problem shape: B=32, S=512, N_Q=1024, N_CATS=4, EMB=64,
ATTN=32, HEADS=4 (head_dim 8).
"""

import os
import sys
from contextlib import ExitStack

import numpy as np

for _p in ("/opt/trn_rl_repo", "/root/.axon_site/_ro/trn_rl_repo"):
    if os.path.isdir(_p) and _p not in sys.path:
        sys.path.append(_p)

import ml_dtypes  # noqa: E402

import concourse.bass as bass  # noqa: E402
import concourse.tile as tile  # noqa: E402
from concourse import bacc, mybir  # noqa: E402
from concourse.bass import IndirectOffsetOnAxis  # noqa: E402
from concourse.bass_utils import run_bass_kernel_spmd  # noqa: E402
from concourse.masks import make_identity  # noqa: E402

BF16 = ml_dtypes.bfloat16
F32 = mybir.dt.float32
BF = mybir.dt.bfloat16
I32 = mybir.dt.int32
ALU = mybir.AluOpType
ACTF = mybir.ActivationFunctionType

B, S, EMB, ATTN, HEADS, HD, C, Q = 32, 512, 64, 32, 4, 8, 4, 1024
NCORES = 8
NB = B // NCORES          # batches per core = 4
NJ = NB * (S // 128)      # token tiles per core = 16
SCALE = 1.0 / np.sqrt(HD)


def build_kernel(nc: bacc.Bacc, tc: tile.TileContext, io: dict):
    """Emit the per-core program. io maps names -> DRAM APs."""
    ctx = ExitStack()
    with ctx:
        _build(nc, tc, ctx, io)


def _build(nc, tc, ctx, io):
    const = ctx.enter_context(tc.tile_pool(name="const", bufs=1))
    sb = ctx.enter_context(tc.tile_pool(name="sb", bufs=2))
    expp = ctx.enter_context(tc.tile_pool(name="expp", bufs=16))
    vsp = ctx.enter_context(tc.tile_pool(name="vsp", bufs=8))
    gp = ctx.enter_context(tc.tile_pool(name="gp", bufs=16))
    big = ctx.enter_context(tc.tile_pool(name="big", bufs=1))
    ps_scores = ctx.enter_context(tc.tile_pool(name="ps_scores", bufs=2, space="PSUM"))
    ps_av = ctx.enter_context(tc.tile_pool(name="ps_av", bufs=1, space="PSUM"))
    ps_sum = ctx.enter_context(tc.tile_pool(name="ps_sum", bufs=1, space="PSUM"))
    ps_misc = ctx.enter_context(tc.tile_pool(name="ps_misc", bufs=2, space="PSUM"))

    # ---------------- constants / weights into SBUF ----------------
    def load_const(name, part, free, dtype):
        t = const.tile([part, free], dtype, tag=name)
        nc.sync.dma_start(out=t[:, :], in_=io[name][:, :])
        return t

    # context loads first on the sync queue: the whole attention pipeline
    # waits on these, and the constant loads below can trail them.
    ceb_l = {}
    for b in range(NB):
        ceb = sb.tile([128, 4 * EMB], F32, tag="ceb")
        nc.sync.dma_start(
            out=ceb[:, :].rearrange("p (cc e) -> p cc e", cc=4),
            in_=io["ce"][b, :, :].rearrange("(cc p) e -> p cc e", p=128),
        )
        ceb_l[b] = ceb

    wctx = load_const("wctx", ATTN, EMB, BF)            # (32,64)
    bctx_bf = load_const("bctx_bf", ATTN, 1, BF)        # (32,1)
    wq_sp = load_const("wq_sp", ATTN, 128, BF)          # (32,128) head h at cols 32h..32h+8, 0 pad
    wk_sp = load_const("wk_sp", ATTN, 128, BF)
    bq_sp = load_const("bq_sp", 128, 1, F32)
    bk_sp = load_const("bk_sp", 128, 1, F32)
    wvT_sp = load_const("wvT_sp", ATTN + 1, 128, BF)    # (33,128) spread cols, last row = b_v
    wout_sp = load_const("wout_sp", 128, ATTN, BF)      # (128,32) spread layout
    bout = load_const("bout", ATTN, 1, F32)
    wsupT_aug = load_const("wsupT_aug", ATTN + 1, C, BF)  # (33,4) last row = b_sup
    bemb_bc = load_const("bemb_bc", 128, EMB, F32)      # (128,64) replicated
    temp = load_const("temp", 1, HEADS, F32)            # (1,4)
    qidx = load_const("qidx", 128, NJ, I32)             # (128,16) token-tiled
    rdat = load_const("rdat", 128, NJ, I32)

    ident = const.tile([128, 128], F32, tag="ident")
    make_identity(nc, ident[:, :])
    ones1 = const.tile([1, 128], F32, tag="ones1")
    nc.vector.memset(ones1[:, :], 1.0)
    ones_bf = const.tile([128, ATTN], BF, tag="ones_bf")
    nc.vector.memset(ones_bf[:, :], 1.0)

    # PE warm-up: ~4.5us of dense matmuls at kernel start trips the HAM
    # activity window so the whole kernel runs at 2.4 GHz instead of 1.2.
    warm = const.tile([128, 128], BF, tag="warm")
    nc.vector.memset(warm[:, :], 0.5)
    warm_ps = ps_misc.tile([128, 128], F32, tag="misc", name="warm_ps")
    for _ in range(28):
        nc.tensor.matmul(warm_ps[0:32, :], warm[:, 0:32], warm[:, :],
                         start=True, stop=True)

    # Fold the ctx projection into q/k/v on device (one-time):
    #   q = ce @ (Wctx^T Wq) + (Wq^T bctx + bq), etc.
    # Removes one full matmul+evac stage from every batch's critical chain.
    wcq_ps = ps_misc.tile([EMB, 128], F32, tag="misc", name="wcq_ps")
    nc.tensor.matmul(wcq_ps[:, :], wctx[:, :], wq_sp[:, :], start=True, stop=True)
    wcq = const.tile([EMB, 128], BF, tag="wcq")
    nc.vector.tensor_copy(wcq[:, :], wcq_ps[:, :])
    wck_ps = ps_misc.tile([EMB, 128], F32, tag="misc", name="wck_ps")
    nc.tensor.matmul(wck_ps[:, :], wctx[:, :], wk_sp[:, :], start=True, stop=True)
    wck = const.tile([EMB, 128], BF, tag="wck")
    nc.vector.tensor_copy(wck[:, :], wck_ps[:, :])
    wcv = const.tile([EMB + 1, 128], BF, tag="wcv")
    wcv_ps = ps_misc.tile([EMB, 128], F32, tag="misc", name="wcv_ps")
    nc.tensor.matmul(wcv_ps[:, :], wctx[:, :], wvT_sp[0:ATTN, :], start=True, stop=True)
    nc.vector.tensor_copy(wcv[0:EMB, :], wcv_ps[:, :])
    wcvb_ps = ps_misc.tile([1, 128], F32, tag="misc", name="wcvb_ps")
    nc.tensor.matmul(wcvb_ps[:, :], bctx_bf[:, :], wvT_sp[0:ATTN, :], start=True, stop=True)
    nc.vector.tensor_tensor(wcv[EMB : EMB + 1, :], wcvb_ps[:, :],
                            wvT_sp[ATTN : ATTN + 1, :], op=ALU.add)
    bq2_ps = ps_misc.tile([128, 1], F32, tag="misc", name="bq2_ps")
    nc.tensor.matmul(bq2_ps[:, :], wq_sp[:, :], bctx_bf[:, :], start=True, stop=True)
    bq2 = const.tile([128, 1], F32, tag="bq2")
    nc.vector.tensor_tensor(bq2[:, :], bq2_ps[:, :], bq_sp[:, :], op=ALU.add)
    bk2_ps = ps_misc.tile([128, 1], F32, tag="misc", name="bk2_ps")
    nc.tensor.matmul(bk2_ps[:, :], wk_sp[:, :], bctx_bf[:, :], start=True, stop=True)
    bk2 = const.tile([128, 1], F32, tag="bk2")
    nc.vector.tensor_tensor(bk2[:, :], bk2_ps[:, :], bk_sp[:, :], op=ALU.add)

    # iota over cats: (128,16) value = i % 4  (i = 4h + c)
    iota_i = const.tile([128, HEADS * C], I32, tag="iota_i")
    nc.gpsimd.iota(
        iota_i[:, :].rearrange("p (h c) -> p h c", c=C),
        pattern=[[0, HEADS], [1, C]],
        channel_multiplier=0,
    )
    kkf = const.tile([128, HEADS * C], F32, tag="kkf")
    nc.vector.tensor_copy(kkf[:, :], iota_i[:, :])

    # 1/T pattern (1,16) then broadcast to (128,16) via PE
    recipT = const.tile([1, HEADS], F32, tag="recipT")
    nc.vector.reciprocal(recipT[:, :], temp[:, :])
    recipT16 = const.tile([1, HEADS * C], F32, tag="recipT16")
    nc.vector.tensor_copy(
        recipT16[:, :].rearrange("p (h c) -> p h c", c=C),
        recipT[:, :].to_broadcast([1, HEADS, C]),
    )
    rt_ps = ps_misc.tile([128, HEADS * C], F32, tag="misc")
    nc.tensor.matmul(rt_ps[:, :], ones1[:, :], recipT16[:, :], start=True, stop=True)
    rt_bc = const.tile([128, HEADS * C], F32, tag="rt_bc")
    nc.vector.tensor_copy(rt_bc[:, :], rt_ps[:, :])

    # ---------------- sharpened (ordinal softmax) path ----------------
    # tokens: partition p, tile j (j = 4*b + cc); free layout i = 4h + c
    rdf = const.tile([128, NJ], F32, tag="rdf")
    nc.vector.tensor_copy(rdf[:, :], rdat[:, :])
    dmat = big.tile([128, NJ * HEADS * C], F32, tag="dmat")
    d3 = dmat[:, :].rearrange("p (j i) -> p j i", i=HEADS * C)
    nc.vector.tensor_tensor(
        d3,
        kkf[:, None, :].to_broadcast([128, NJ, HEADS * C]),
        rdf[:, :, None].to_broadcast([128, NJ, HEADS * C]),
        op=ALU.subtract,
    )
    # a = -|d|/3   (|d| = max(d, -d))
    ndmat = big.tile([128, NJ * HEADS * C], F32, tag="ndmat")
    nc.vector.tensor_scalar_mul(ndmat[:, :], dmat[:, :], -1.0)
    nc.vector.tensor_tensor(dmat[:, :], dmat[:, :], ndmat[:, :], op=ALU.max)
    nc.vector.tensor_scalar_mul(dmat[:, :], dmat[:, :], -1.0 / (C - 1))
    # bw = relu(1 + a)
    nc.scalar.activation(dmat[:, :], dmat[:, :], ACTF.Relu, bias=1.0, scale=1.0)
    # e_in = bw * (1/T_h)
    nc.vector.tensor_tensor(
        d3,
        d3,
        rt_bc[:, None, :].to_broadcast([128, NJ, HEADS * C]),
        op=ALU.mult,
    )
    nc.scalar.activation(dmat[:, :], dmat[:, :], ACTF.Exp)
    sums2 = big.tile([128, NJ * HEADS], F32, tag="sums2")
    nc.vector.tensor_reduce(
        sums2[:, :],
        dmat[:, :].rearrange("p (a c) -> p a c", c=C),
        axis=mybir.AxisListType.X,
        op=ALU.add,
    )
    r2 = big.tile([128, NJ * HEADS], F32, tag="r2")
    nc.vector.reciprocal(r2[:, :], sums2[:, :])
    p2 = big.tile([128, NJ * HEADS * C], F32, tag="p2")
    nc.vector.scalar_tensor_tensor(
        p2[:, :].rearrange("p (a c) -> p a c", c=C),
        dmat[:, :].rearrange("p (a c) -> p a c", c=C),
        0.125,
        r2[:, :, None].to_broadcast([128, NJ * HEADS, C]),
        op0=ALU.mult,
        op1=ALU.mult,
    )
    sharp = big.tile([128, NJ * C], F32, tag="sharp")
    nc.vector.tensor_reduce(
        sharp[:, :].rearrange("p (j c) -> p j c", c=C),
        p2[:, :].rearrange("p (j h c) -> p j c h", h=HEADS, c=C),
        axis=mybir.AxisListType.X,
        op=ALU.add,
    )

    # ---------------- gathers (independent of attention) ----------------
    # G_all free layout: j (16) x c (4) x e (64); GJ tokens per indirect DMA.
    g_all = big.tile([128, NJ * C * EMB], F32, tag="g_all")
    for j in range(NJ):
        nc.gpsimd.indirect_dma_start(
            out=g_all[:, C * EMB * j : C * EMB * (j + 1)],
            out_offset=None,
            in_=io["w3T"][:, :],
            in_offset=IndirectOffsetOnAxis(ap=qidx[:, j : j + 1], axis=0),
        )

    fw = big.tile([128, NJ * C], F32, tag="fw")
    out_all = big.tile([128, NJ * EMB], F32, tag="out_all")
    pmat = big.tile([128, NJ * C * EMB], F32, tag="pmat")

    # ---------------- per-batch attention, staged ----------------
    ceT_l, ctxT_l, qs_l, ks_l, v_l = {}, {}, {}, {}, {}

    def stage_transpose(b):
        ceb = ceb_l[b]
        ceT_ps = ps_misc.tile([EMB, S], F32, tag="misc", name="ceT_ps")
        for cc in range(4):
            nc.tensor.transpose(
                ceT_ps[:, 128 * cc : 128 * (cc + 1)],
                ceb[:, EMB * cc : EMB * (cc + 1)],
                ident[:, :],
            )
        ceT = sb.tile([EMB + 1, S], BF, tag="ceT", name="ceT")
        nc.vector.tensor_copy(ceT[0:EMB, :], ceT_ps[:, :])
        nc.vector.memset(ceT[EMB : EMB + 1, :], 1.0)
        ceT_l[b] = ceT

    def stage_qk(b):
        ceT = ceT_l[b]
        qs_ps = ps_misc.tile([128, S], F32, tag="misc", name="qs_ps")
        for h in range(HEADS):
            nc.tensor.matmul(
                qs_ps[32 * h : 32 * (h + 1), :],
                wcq[:, 32 * h : 32 * (h + 1)],
                ceT[0:EMB, :],
                start=True, stop=True,
                tile_position=(0, 32 * h),
            )
        qs = sb.tile([128, S], BF, tag="qs", name="qs")
        nc.vector.tensor_scalar_add(qs[:, :], qs_ps[:, :], bq2[:, :])
        qs_l[b] = qs
        ks_ps = ps_misc.tile([128, S], F32, tag="misc", name="ks_ps")
        for h in range(HEADS):
            nc.tensor.matmul(
                ks_ps[32 * h : 32 * (h + 1), :],
                wck[:, 32 * h : 32 * (h + 1)],
                ceT[0:EMB, :],
                start=True, stop=True,
                tile_position=(0, 32 * h),
            )
        ks = sb.tile([128, S], BF, tag="ks", name="ks")
        nc.vector.tensor_scalar_add(ks[:, :], ks_ps[:, :], bk2[:, :])
        ks_l[b] = ks

    def stage_v(b):
        ceT = ceT_l[b]
        v_ps = ps_misc.tile([128, S], F32, tag="misc", name="v_ps")
        for cc in range(4):
            nc.tensor.matmul(
                v_ps[:, 128 * cc : 128 * (cc + 1)],
                ceT[:, 128 * cc : 128 * (cc + 1)],
                wcv[:, :],
                start=True, stop=True,
            )
        v_sp = vsp.tile([128, S], BF, tag="v_sp", name="v_sp")
        nc.vector.tensor_copy(v_sp[:, :], v_ps[:, :])
        v_l[b] = v_sp

    A_STAGES = [stage_transpose, stage_qk, stage_v, lambda b: None]

    def phase_a(b):
        for f in A_STAGES:
            f(b)

    # ---- scores/exp/AV software pipeline: batch b's QK waves interleave
    # with batch b-1's AV+sums waves so the PE stream stays dense ----
    NBJ = 4  # token tiles per batch

    def qk_wave(b, cc):
        qs, ks = qs_l[b], ks_l[b]
        ets = []
        tiles = []
        for hh in range(2):  # head halves
            sc_ps = ps_scores.tile([128, 2 * S], F32, tag="scores")
            for hi in range(2):
                h = 2 * hh + hi
                for jj in range(4):
                    nc.tensor.matmul(
                        sc_ps[32 * jj : 32 * (jj + 1), S * hi : S * (hi + 1)],
                        ks[32 * h : 32 * h + HD,
                           128 * cc + 32 * jj : 128 * cc + 32 * (jj + 1)],
                        qs[32 * h : 32 * h + HD, :],
                        start=True,
                        stop=True,
                        tile_position=(32 * h, 32 * jj),
                    )
            tiles.append(sc_ps)
        for sc_ps in tiles:
            et = expp.tile([128, 2 * S], BF, tag="expT")
            nc.scalar.activation(et[:, :], sc_ps[:, :], ACTF.Exp, scale=SCALE)
            ets.append(et)
        return ets

    def av_wave(b, cc, avt_ps, sums_ps, ets):
        for h in range(HEADS):
            mv = ets[h // 2][:, S * (h % 2) : S * (h % 2 + 1)]
            nc.tensor.matmul(
                avt_ps[32 * h : 32 * (h + 1), :],
                v_l[b][:, 128 * cc + 32 * h : 128 * cc + 32 * (h + 1)],
                mv,
                start=(cc == 0),
                stop=(cc == 3),
                tile_position=(0, 32 * h),
                skip_group_check=True,
            )
        for h in range(HEADS):
            mv = ets[h // 2][:, S * (h % 2) : S * (h % 2 + 1)]
            nc.tensor.matmul(
                sums_ps[32 * h : 32 * (h + 1), :],
                ones_bf[:, :],
                mv,
                start=(cc == 0),
                stop=(cc == 3),
                tile_position=(0, 32 * h),
                skip_group_check=True,
            )

    def post_batch(b, avt_ps, sums_ps):
        rec = sb.tile([128, S], F32, tag="rec")
        nc.vector.reciprocal_approx_fast(rec[:, :], sums_ps[:, :])
        normT = sb.tile([128, S], BF, tag="normT")
        nc.vector.tensor_tensor(normT[:, :], avt_ps[:, :], rec[:, :], op=ALU.mult)

        # O^T = W_out_spread.T @ normT + b_out  -> (33,512) with ones row
        o_ps = ps_misc.tile([ATTN, S], F32, tag="misc")
        nc.tensor.matmul(o_ps[:, :], wout_sp[:, :], normT[:, :], start=True, stop=True)
        oT = sb.tile([ATTN + 1, S], BF, tag="oT")
        nc.vector.tensor_scalar_add(oT[0:ATTN, :], o_ps[:, :], bout[:, :])
        nc.vector.memset(oT[ATTN : ATTN + 1, :], 1.0)

        # suppression logits z: (128, 16) free = 4*cc + c
        sup_ps = ps_misc.tile([128, 4 * C], F32, tag="misc")
        for cc in range(4):
            nc.tensor.matmul(
                sup_ps[:, C * cc : C * (cc + 1)],
                oT[:, 128 * cc : 128 * (cc + 1)],
                wsupT_aug[:, :],
                start=True,
                stop=True,
            )
        # 1 + sigmoid(-z) = (2 + e^z) / (1 + e^z); keeps ACT on the exp table
        tb = sb.tile([128, 4 * C], F32, tag="tb")
        nc.scalar.activation(tb[:, :], sup_ps[:, :], ACTF.Exp)
        ab = sb.tile([128, 4 * C], F32, tag="ab")
        nc.vector.tensor_scalar_add(ab[:, :], tb[:, :], 1.0)
        rb = sb.tile([128, 4 * C], F32, tag="rb")
        nc.vector.reciprocal_approx_fast(rb[:, :], ab[:, :])
        ub = sb.tile([128, 4 * C], F32, tag="ub")
        nc.vector.scalar_tensor_tensor(
            ub[:, :], tb[:, :], 2.0, rb[:, :], op0=ALU.add, op1=ALU.mult
        )
        # fw = (1 + sigmoid(-z)) * sharp  (0.5 mean+suppression folded in sharp)
        nc.vector.tensor_tensor(
            fw[:, 16 * b : 16 * (b + 1)],
            ub[:, :],
            sharp[:, 16 * b : 16 * (b + 1)],
            op=ALU.mult,
        )

        # per-batch final gather-contract (3 DVE ops) + store
        gsl = g_all[:, C * EMB * NBJ * b : C * EMB * NBJ * (b + 1)]
        pm = pmat[:, C * EMB * NBJ * b : C * EMB * NBJ * (b + 1)]
        osl = out_all[:, EMB * NBJ * b : EMB * NBJ * (b + 1)]
        nc.vector.tensor_tensor(
            pm.rearrange("p (j c e) -> p j c e", c=C, e=EMB),
            gsl.rearrange("p (j c e) -> p j c e", c=C, e=EMB),
            fw[:, 16 * b : 16 * (b + 1)].rearrange("p (j c) -> p j c", c=C)[
                :, :, :, None
            ].to_broadcast([128, NBJ, C, EMB]),
            op=ALU.mult,
        )
        nc.vector.tensor_reduce(
            osl.rearrange("p (j e) -> p j e", e=EMB),
            pm.rearrange("p (j c e) -> p j e c", c=C, e=EMB),
            axis=mybir.AxisListType.X,
            op=ALU.add,
        )
        nc.vector.tensor_tensor(
            osl.rearrange("p (j e) -> p j e", e=EMB),
            osl.rearrange("p (j e) -> p j e", e=EMB),
            bemb_bc[:, None, :].to_broadcast([128, NBJ, EMB]),
            op=ALU.add,
        )
        nc.sync.dma_start(
            out=io["out"][b, :, :].rearrange("(cc p) e -> p cc e", p=128),
            in_=osl.rearrange("p (cc e) -> p cc e", cc=4),
        )

    ets_l = {b: [None] * 4 for b in range(NB)}
    av_tiles = {}

    def get_av(b):
        if b not in av_tiles:
            av_tiles[b] = (
                ps_av.tile([128, S], F32, tag="avt", name="avt_ps"),
                ps_sum.tile([128, S], F32, tag="sums", name="sums_ps"),
            )
        return av_tiles[b]

    phase_a(0)
    prev = None  # (b, cc) whose AV wave is pending, 1 step behind
    for b in range(NB):
        for cc in range(4):
            ets_l[b][cc] = qk_wave(b, cc)
            if b + 1 < NB:
                A_STAGES[cc](b + 1)
            if prev is not None:
                pb, pcc = prev
                av_wave(pb, pcc, *get_av(pb), ets_l[pb][pcc])
                if pcc == 3:
                    post_batch(pb, *av_tiles[pb])
            prev = (b, cc)
    av_wave(NB - 1, 3, *get_av(NB - 1), ets_l[NB - 1][3])
    post_batch(NB - 1, *av_tiles[NB - 1])


# ======================= host side =======================

def _prep_weights(inp):
    """Pure layout transforms of the parameters (shared by all cores)."""
    f32 = np.float32

    def bf(x):
        return np.ascontiguousarray(np.asarray(x, f32).astype(BF16))

    W_ctx = np.asarray(inp["W_ctx"], f32)
    W_in = np.asarray(inp["W_in"], f32)
    W_out = np.asarray(inp["W_out"], f32)
    W_sup = np.asarray(inp["W_sup"], f32)
    W_emb = np.asarray(inp["W_emb"], f32)
    b_ctx = np.asarray(inp["b_ctx"], f32)
    b_in = np.asarray(inp["b_in"], f32)
    b_out = np.asarray(inp["b_out"], f32)
    b_sup = np.asarray(inp["b_sup"], f32)
    b_emb = np.asarray(inp["b_emb"], f32)
    temp = np.asarray(inp["temperature"], f32)

    w = {}
    w["wctx"] = bf(W_ctx)                                      # (32,64)
    w["bctx_bf"] = bf(b_ctx[:, None])                          # (32,1)
    wq = np.zeros((ATTN, 128), f32)
    wk = np.zeros((ATTN, 128), f32)
    bq = np.zeros((128, 1), f32)
    bk = np.zeros((128, 1), f32)
    for h in range(HEADS):
        wq[:, 32 * h : 32 * h + HD] = W_in[HD * h : HD * (h + 1), :].T
        wk[:, 32 * h : 32 * h + HD] = W_in[ATTN + HD * h : ATTN + HD * (h + 1), :].T
        bq[32 * h : 32 * h + HD, 0] = b_in[HD * h : HD * (h + 1)]
        bk[32 * h : 32 * h + HD, 0] = b_in[ATTN + HD * h : ATTN + HD * (h + 1)]
    w["wq_sp"], w["wk_sp"], w["bq_sp"], w["bk_sp"] = bf(wq), bf(wk), bq, bk
    wv = np.zeros((ATTN + 1, 128), f32)
    for h in range(HEADS):
        wv[0:ATTN, 32 * h : 32 * h + HD] = W_in[2 * ATTN + HD * h : 2 * ATTN + HD * (h + 1), :].T
        wv[ATTN, 32 * h : 32 * h + HD] = b_in[2 * ATTN + HD * h : 2 * ATTN + HD * (h + 1)]
    w["wvT_sp"] = bf(wv)
    wout = np.zeros((128, ATTN), f32)
    for h in range(HEADS):
        wout[32 * h : 32 * h + HD, :] = W_out[:, HD * h : HD * (h + 1)].T
    w["wout_sp"] = bf(wout)
    w["bout"] = np.ascontiguousarray(b_out[:, None])
    w["wsupT_aug"] = bf(np.concatenate([W_sup.T, b_sup[None, :]], 0))  # (33,4)
    w["bemb_bc"] = np.ascontiguousarray(np.broadcast_to(b_emb[None, :], (128, EMB)))
    w["temp"] = np.ascontiguousarray(temp[None, :])
    # W3T[q, c*64+e] = W_emb[e, c*Q+q]  (pure transpose)
    w["w3T"] = np.ascontiguousarray(
        W_emb.reshape(EMB, C, Q).transpose(2, 1, 0).reshape(Q, C * EMB)
    )
    return w


def _spec():
    """name -> (shape, mybir dtype) for all per-core DRAM tensors."""
    return {
        "wctx": ((ATTN, EMB), BF), "bctx_bf": ((ATTN, 1), BF),
        "wq_sp": ((ATTN, 128), BF), "wk_sp": ((ATTN, 128), BF),
        "bq_sp": ((128, 1), F32), "bk_sp": ((128, 1), F32),
        "wvT_sp": ((ATTN + 1, 128), BF),
        "wout_sp": ((128, ATTN), BF), "bout": ((ATTN, 1), F32),
        "wsupT_aug": ((ATTN + 1, C), BF),
        "bemb_bc": ((128, EMB), F32), "temp": ((1, HEADS), F32),
        "qidx": ((128, NJ), I32), "rdat": ((128, NJ), I32),
        "w3T": ((Q, C * EMB), F32),
        "ce": ((NB, S, EMB), F32),
    }


def build_bass():
    nc = bacc.Bacc("TRN2", target_bir_lowering=False, debug=False)
    io = {}
    for name, (shape, dt) in _spec().items():
        io[name] = nc.dram_tensor(name, list(shape), dt, kind="ExternalInput").ap()
    io["out"] = nc.dram_tensor("out", [NB, S, EMB], F32, kind="ExternalOutput").ap()
    with tile.TileContext(nc) as tc:
        build_kernel(nc, tc, io)
    nc.compile()
    return nc


def make_in_maps(inputs):
    inp = dict(inputs)
    w = _prep_weights(inp)
    q_idx = np.asarray(inp["q_idx"]).astype(np.int32)
    r_data = np.asarray(inp["r_data"]).astype(np.int32)
    ce = np.asarray(inp["context_embedding"], np.float32)

    in_maps = []
    for k in range(NCORES):
        m = dict(w)
        qs = q_idx[NB * k : NB * (k + 1)]          # (4,512)
        rs = r_data[NB * k : NB * (k + 1)]
        # token-tile layout: [p, j] with j = 4*b + cc, s = 128*cc + p
        m["qidx"] = np.ascontiguousarray(
            qs.reshape(NB, 4, 128).transpose(2, 0, 1).reshape(128, NJ)
        )
        m["rdat"] = np.ascontiguousarray(
            rs.reshape(NB, 4, 128).transpose(2, 0, 1).reshape(128, NJ)
        )
        m["ce"] = np.ascontiguousarray(ce[NB * k : NB * (k + 1)])
        in_maps.append(m)
    return in_maps


_NC_CACHE = {}


def kernel(**inputs) -> np.ndarray:
    if "nc" not in _NC_CACHE:
        _NC_CACHE["nc"] = build_bass()
    nc = _NC_CACHE["nc"]
    in_maps = make_in_maps(inputs)
    res = run_bass_kernel_spmd(nc, in_maps, core_ids=list(range(NCORES)))
    out = np.concatenate([res.results[k]["out"] for k in range(NCORES)], axis=0)
    return out.astype(np.float32)



# revision 8
# speedup vs baseline: 1.0676x; 1.0676x over previous
"""Trainium2 Bass kernel for nn_AttentionModulatedOrdinalEmbedding.

Contract: kernel(**inputs) takes the FULL (unsharded) inputs from
setup_inputs() and returns the FULL (B, S, EMB) float32 output.
Internally shards batch-parallel across 8 NeuronCores (4 batches/core),
runs one SPMD Bass kernel, and concatenates the per-core outputs.

Hardcoded problem shape: B=32, S=512, N_Q=1024, N_CATS=4, EMB=64,
ATTN=32, HEADS=4 (head_dim 8).
"""

import os
import sys
from contextlib import ExitStack

import numpy as np

for _p in ("/opt/trn_rl_repo", "/root/.axon_site/_ro/trn_rl_repo"):
    if os.path.isdir(_p) and _p not in sys.path:
        sys.path.append(_p)

import ml_dtypes  # noqa: E402

import concourse.bass as bass  # noqa: E402
import concourse.tile as tile  # noqa: E402
from concourse import bacc, mybir  # noqa: E402
from concourse.bass import IndirectOffsetOnAxis  # noqa: E402
from concourse.bass_utils import run_bass_kernel_spmd  # noqa: E402
from concourse.masks import make_identity  # noqa: E402

BF16 = ml_dtypes.bfloat16
F32 = mybir.dt.float32
BF = mybir.dt.bfloat16
I32 = mybir.dt.int32
ALU = mybir.AluOpType
ACTF = mybir.ActivationFunctionType

B, S, EMB, ATTN, HEADS, HD, C, Q = 32, 512, 64, 32, 4, 8, 4, 1024
NCORES = 8
NB = B // NCORES          # batches per core = 4
NJ = NB * (S // 128)      # token tiles per core = 16
SCALE = 1.0 / np.sqrt(HD)


def build_kernel(nc: bacc.Bacc, tc: tile.TileContext, io: dict):
    """Emit the per-core program. io maps names -> DRAM APs."""
    ctx = ExitStack()
    with ctx:
        _build(nc, tc, ctx, io)


def _build(nc, tc, ctx, io):
    const = ctx.enter_context(tc.tile_pool(name="const", bufs=1))
    sb = ctx.enter_context(tc.tile_pool(name="sb", bufs=2))
    expp = ctx.enter_context(tc.tile_pool(name="expp", bufs=6))
    vsp = ctx.enter_context(tc.tile_pool(name="vsp", bufs=8))
    big = ctx.enter_context(tc.tile_pool(name="big", bufs=1))
    ps_scA = ctx.enter_context(tc.tile_pool(name="ps_scA", bufs=1, space="PSUM"))
    ps_scB = ctx.enter_context(tc.tile_pool(name="ps_scB", bufs=1, space="PSUM"))
    ps_av = ctx.enter_context(tc.tile_pool(name="ps_av", bufs=1, space="PSUM"))
    ps_sum = ctx.enter_context(tc.tile_pool(name="ps_sum", bufs=1, space="PSUM"))
    ps_misc = ctx.enter_context(tc.tile_pool(name="ps_misc", bufs=2, space="PSUM"))

    # ---------------- constants / weights into SBUF ----------------
    def load_const(name, part, free, dtype):
        t = const.tile([part, free], dtype, tag=name)
        nc.sync.dma_start(out=t[:, :], in_=io[name][:, :])
        return t

    # context loads first on the sync queue: the whole attention pipeline
    # waits on these, and the constant loads below can trail them.
    ceb_l = {}
    for b in range(NB):
        ceb = sb.tile([128, 4 * EMB], F32, tag="ceb")
        nc.sync.dma_start(
            out=ceb[:, :].rearrange("p (cc e) -> p cc e", cc=4),
            in_=io["ce"][b, :, :].rearrange("(cc p) e -> p cc e", p=128),
        )
        ceb_l[b] = ceb

    wctx = load_const("wctx", ATTN, EMB, BF)            # (32,64)
    bctx_bf = load_const("bctx_bf", ATTN, 1, BF)        # (32,1)
    wq_sp = load_const("wq_sp", ATTN, 128, BF)          # (32,128) head h at cols 32h..32h+8, 0 pad
    wk_sp = load_const("wk_sp", ATTN, 128, BF)
    bq_sp = load_const("bq_sp", 128, 1, F32)
    bk_sp = load_const("bk_sp", 128, 1, F32)
    wvT_sp = load_const("wvT_sp", ATTN + 1, 128, BF)    # (33,128) spread cols, last row = b_v
    wout_sp = load_const("wout_sp", 128, ATTN, BF)      # (128,32) spread layout
    bout = load_const("bout", ATTN, 1, F32)
    wsupT_aug = load_const("wsupT_aug", ATTN + 1, C, BF)  # (33,4) last row = b_sup
    bemb_bc = load_const("bemb_bc", 128, EMB, F32)      # (128,64) replicated
    temp = load_const("temp", 1, HEADS, F32)            # (1,4)
    qidx = load_const("qidx", 128, NJ, I32)             # (128,16) token-tiled
    rdat = load_const("rdat", 128, NJ, I32)

    ident = const.tile([128, 128], F32, tag="ident")
    make_identity(nc, ident[:, :])
    ones1 = const.tile([1, 128], F32, tag="ones1")
    nc.vector.memset(ones1[:, :], 1.0)
    ones_bf = const.tile([128, ATTN], BF, tag="ones_bf")
    nc.vector.memset(ones_bf[:, :], 1.0)

    # PE warm-up: ~4.5us of dense matmuls at kernel start trips the HAM
    # activity window so the whole kernel runs at 2.4 GHz instead of 1.2.
    warm = const.tile([128, 128], BF, tag="warm")
    nc.vector.memset(warm[:, :], 0.5)
    warm_ps = ps_misc.tile([128, 128], F32, tag="misc", name="warm_ps")
    for _ in range(28):
        nc.tensor.matmul(warm_ps[0:32, :], warm[:, 0:32], warm[:, :],
                         start=True, stop=True)

    # Fold the ctx projection into q/k/v on device (one-time):
    #   q = ce @ (Wctx^T Wq) + (Wq^T bctx + bq), etc.
    # Removes one full matmul+evac stage from every batch's critical chain.
    wcq_ps = ps_misc.tile([EMB, 128], F32, tag="misc", name="wcq_ps")
    nc.tensor.matmul(wcq_ps[:, :], wctx[:, :], wq_sp[:, :], start=True, stop=True)
    wcq = const.tile([EMB, 128], BF, tag="wcq")
    nc.vector.tensor_copy(wcq[:, :], wcq_ps[:, :])
    wck_ps = ps_misc.tile([EMB, 128], F32, tag="misc", name="wck_ps")
    nc.tensor.matmul(wck_ps[:, :], wctx[:, :], wk_sp[:, :], start=True, stop=True)
    wck = const.tile([EMB, 128], BF, tag="wck")
    nc.vector.tensor_copy(wck[:, :], wck_ps[:, :])
    wcv = const.tile([EMB + 1, 128], BF, tag="wcv")
    wcv_ps = ps_misc.tile([EMB, 128], F32, tag="misc", name="wcv_ps")
    nc.tensor.matmul(wcv_ps[:, :], wctx[:, :], wvT_sp[0:ATTN, :], start=True, stop=True)
    nc.vector.tensor_copy(wcv[0:EMB, :], wcv_ps[:, :])
    wcvb_ps = ps_misc.tile([1, 128], F32, tag="misc", name="wcvb_ps")
    nc.tensor.matmul(wcvb_ps[:, :], bctx_bf[:, :], wvT_sp[0:ATTN, :], start=True, stop=True)
    nc.vector.tensor_tensor(wcv[EMB : EMB + 1, :], wcvb_ps[:, :],
                            wvT_sp[ATTN : ATTN + 1, :], op=ALU.add)
    bq2_ps = ps_misc.tile([128, 1], F32, tag="misc", name="bq2_ps")
    nc.tensor.matmul(bq2_ps[:, :], wq_sp[:, :], bctx_bf[:, :], start=True, stop=True)
    bq2 = const.tile([128, 1], F32, tag="bq2")
    nc.vector.tensor_tensor(bq2[:, :], bq2_ps[:, :], bq_sp[:, :], op=ALU.add)
    bk2_ps = ps_misc.tile([128, 1], F32, tag="misc", name="bk2_ps")
    nc.tensor.matmul(bk2_ps[:, :], wk_sp[:, :], bctx_bf[:, :], start=True, stop=True)
    bk2 = const.tile([128, 1], F32, tag="bk2")
    nc.vector.tensor_tensor(bk2[:, :], bk2_ps[:, :], bk_sp[:, :], op=ALU.add)

    # iota over cats: (128,16) value = i % 4  (i = 4h + c)
    iota_i = const.tile([128, HEADS * C], I32, tag="iota_i")
    nc.gpsimd.iota(
        iota_i[:, :].rearrange("p (h c) -> p h c", c=C),
        pattern=[[0, HEADS], [1, C]],
        channel_multiplier=0,
    )
    kkf = const.tile([128, HEADS * C], F32, tag="kkf")
    nc.vector.tensor_copy(kkf[:, :], iota_i[:, :])

    # 1/T pattern (1,16) then broadcast to (128,16) via PE
    recipT = const.tile([1, HEADS], F32, tag="recipT")
    nc.vector.reciprocal(recipT[:, :], temp[:, :])
    recipT16 = const.tile([1, HEADS * C], F32, tag="recipT16")
    nc.vector.tensor_copy(
        recipT16[:, :].rearrange("p (h c) -> p h c", c=C),
        recipT[:, :].to_broadcast([1, HEADS, C]),
    )
    rt_ps = ps_misc.tile([128, HEADS * C], F32, tag="misc")
    nc.tensor.matmul(rt_ps[:, :], ones1[:, :], recipT16[:, :], start=True, stop=True)
    rt_bc = const.tile([128, HEADS * C], F32, tag="rt_bc")
    nc.vector.tensor_copy(rt_bc[:, :], rt_ps[:, :])

    # ---------------- sharpened (ordinal softmax) path ----------------
    # tokens: partition p, tile j (j = 4*b + cc); free layout i = 4h + c
    rdf = const.tile([128, NJ], F32, tag="rdf")
    nc.vector.tensor_copy(rdf[:, :], rdat[:, :])
    dmat = big.tile([128, NJ * HEADS * C], F32, tag="dmat")
    d3 = dmat[:, :].rearrange("p (j i) -> p j i", i=HEADS * C)
    nc.vector.tensor_tensor(
        d3,
        kkf[:, None, :].to_broadcast([128, NJ, HEADS * C]),
        rdf[:, :, None].to_broadcast([128, NJ, HEADS * C]),
        op=ALU.subtract,
    )
    # a = -|d|/3   (|d| = max(d, -d))
    ndmat = big.tile([128, NJ * HEADS * C], F32, tag="ndmat")
    nc.vector.tensor_scalar_mul(ndmat[:, :], dmat[:, :], -1.0)
    nc.vector.tensor_tensor(dmat[:, :], dmat[:, :], ndmat[:, :], op=ALU.max)
    nc.vector.tensor_scalar_mul(dmat[:, :], dmat[:, :], -1.0 / (C - 1))
    # bw = relu(1 + a)
    nc.scalar.activation(dmat[:, :], dmat[:, :], ACTF.Relu, bias=1.0, scale=1.0)
    # e_in = bw * (1/T_h)
    nc.vector.tensor_tensor(
        d3,
        d3,
        rt_bc[:, None, :].to_broadcast([128, NJ, HEADS * C]),
        op=ALU.mult,
    )
    nc.scalar.activation(dmat[:, :], dmat[:, :], ACTF.Exp)
    sums2 = big.tile([128, NJ * HEADS], F32, tag="sums2")
    nc.vector.tensor_reduce(
        sums2[:, :],
        dmat[:, :].rearrange("p (a c) -> p a c", c=C),
        axis=mybir.AxisListType.X,
        op=ALU.add,
    )
    r2 = big.tile([128, NJ * HEADS], F32, tag="r2")
    nc.vector.reciprocal(r2[:, :], sums2[:, :])
    p2 = big.tile([128, NJ * HEADS * C], F32, tag="p2")
    nc.vector.scalar_tensor_tensor(
        p2[:, :].rearrange("p (a c) -> p a c", c=C),
        dmat[:, :].rearrange("p (a c) -> p a c", c=C),
        0.125,
        r2[:, :, None].to_broadcast([128, NJ * HEADS, C]),
        op0=ALU.mult,
        op1=ALU.mult,
    )
    sharp = big.tile([128, NJ * C], F32, tag="sharp")
    nc.vector.tensor_reduce(
        sharp[:, :].rearrange("p (j c) -> p j c", c=C),
        p2[:, :].rearrange("p (j h c) -> p j c h", h=HEADS, c=C),
        axis=mybir.AxisListType.X,
        op=ALU.add,
    )

    # ---------------- gathers (independent of attention) ----------------
    # G_all free layout: j (16) x e (64) x c (4), bf16: halves HBM traffic
    # and doubles DVE rate; c contiguous so the reduce reads sequentially.
    g_all = big.tile([128, NJ * C * EMB], BF, tag="g_all")
    for j in range(NJ):
        nc.gpsimd.indirect_dma_start(
            out=g_all[:, C * EMB * j : C * EMB * (j + 1)],
            out_offset=None,
            in_=io["w3T"][:, :],
            in_offset=IndirectOffsetOnAxis(ap=qidx[:, j : j + 1], axis=0),
        )

    fw = big.tile([128, NJ * C], F32, tag="fw")
    out_all = big.tile([128, NJ * EMB], F32, tag="out_all")
    pmat = big.tile([128, NJ * C * EMB], BF, tag="pmat")

    # ---------------- per-batch attention, staged ----------------
    ceT_l, ctxT_l, qs_l, ks_l, v_l = {}, {}, {}, {}, {}

    def stage_transpose(b):
        ceb = ceb_l[b]
        ceT_ps = ps_misc.tile([EMB, S], F32, tag="misc", name="ceT_ps")
        for cc in range(4):
            nc.tensor.transpose(
                ceT_ps[:, 128 * cc : 128 * (cc + 1)],
                ceb[:, EMB * cc : EMB * (cc + 1)],
                ident[:, :],
            )
        ceT = sb.tile([EMB + 1, S], BF, tag="ceT", name="ceT")
        nc.vector.tensor_copy(ceT[0:EMB, :], ceT_ps[:, :])
        nc.vector.memset(ceT[EMB : EMB + 1, :], 1.0)
        ceT_l[b] = ceT

    def stage_qk(b):
        ceT = ceT_l[b]
        qs_ps = ps_misc.tile([128, S], F32, tag="misc", name="qs_ps")
        for h in range(HEADS):
            nc.tensor.matmul(
                qs_ps[32 * h : 32 * (h + 1), :],
                wcq[:, 32 * h : 32 * (h + 1)],
                ceT[0:EMB, :],
                start=True, stop=True,
                tile_position=(0, 32 * h),
            )
        qs = sb.tile([128, S], BF, tag="qs", name="qs")
        nc.vector.tensor_scalar_add(qs[:, :], qs_ps[:, :], bq2[:, :])
        qs_l[b] = qs
        ks_ps = ps_misc.tile([128, S], F32, tag="misc", name="ks_ps")
        for h in range(HEADS):
            nc.tensor.matmul(
                ks_ps[32 * h : 32 * (h + 1), :],
                wck[:, 32 * h : 32 * (h + 1)],
                ceT[0:EMB, :],
                start=True, stop=True,
                tile_position=(0, 32 * h),
            )
        ks = sb.tile([128, S], BF, tag="ks", name="ks")
        nc.vector.tensor_scalar_add(ks[:, :], ks_ps[:, :], bk2[:, :])
        ks_l[b] = ks

    def stage_v(b):
        ceT = ceT_l[b]
        v_ps = ps_misc.tile([128, S], F32, tag="misc", name="v_ps")
        for cc in range(4):
            nc.tensor.matmul(
                v_ps[:, 128 * cc : 128 * (cc + 1)],
                ceT[:, 128 * cc : 128 * (cc + 1)],
                wcv[:, :],
                start=True, stop=True,
            )
        v_sp = vsp.tile([128, S], BF, tag="v_sp", name="v_sp")
        nc.vector.tensor_copy(v_sp[:, :], v_ps[:, :])
        v_l[b] = v_sp

    A_STAGES = [stage_transpose, stage_qk, stage_v, lambda b: None]

    def phase_a(b):
        for f in A_STAGES:
            f(b)

    # ---- scores/exp/AV software pipeline: batch b's QK waves interleave
    # with batch b-1's AV+sums waves so the PE stream stays dense ----
    NBJ = 4  # token tiles per batch

    def qk_wave(b, cc):
        # Scores^T for all 4 heads of key-chunk cc: one MM per head at row
        # group 32h (stat = K slice (8,128), mov = Q (8,512)) -> the 4 MMs run
        # concurrently on disjoint row groups, each filling one PSUM bank
        # (128 keys x 512 queries). Heads 0-1 -> tile A, heads 2-3 -> tile B
        # so QK(i+1) into A can overlap exp(i) reading B.
        qs, ks = qs_l[b], ks_l[b]
        scA = ps_scA.tile([128, 2 * S], F32, tag="scA")
        scB = ps_scB.tile([128, 2 * S], F32, tag="scB")
        for h in range(HEADS):
            sc = scA if h < 2 else scB
            nc.tensor.matmul(
                sc[:, S * (h % 2) : S * (h % 2 + 1)],
                ks[32 * h : 32 * h + HD, 128 * cc : 128 * (cc + 1)],
                qs[32 * h : 32 * h + HD, :],
                start=True,
                stop=True,
                tile_position=(32 * h, 0),
            )
        ets = []
        for sc in (scA, scB):
            et = expp.tile([128, 2 * S], BF, tag="expT")
            nc.scalar.activation(et[:, :], sc[:, :], ACTF.Exp, scale=SCALE)
            ets.append(et)
        return ets

    def av_wave(b, cc, avt_ps, sums_ps, ets):
        for h in range(HEADS):
            mv = ets[h // 2][:, S * (h % 2) : S * (h % 2 + 1)]
            nc.tensor.matmul(
                avt_ps[32 * h : 32 * (h + 1), :],
                v_l[b][:, 128 * cc + 32 * h : 128 * cc + 32 * (h + 1)],
                mv,
                start=(cc == 0),
                stop=(cc == 3),
                tile_position=(0, 32 * h),
                skip_group_check=True,
            )
        for h in range(HEADS):
            mv = ets[h // 2][:, S * (h % 2) : S * (h % 2 + 1)]
            nc.tensor.matmul(
                sums_ps[32 * h : 32 * (h + 1), :],
                ones_bf[:, :],
                mv,
                start=(cc == 0),
                stop=(cc == 3),
                tile_position=(0, 32 * h),
                skip_group_check=True,
            )

    def post_batch(b, avt_ps, sums_ps):
        rec = sb.tile([128, S], F32, tag="rec")
        nc.vector.reciprocal_approx_fast(rec[:, :], sums_ps[:, :])
        normT = sb.tile([128, S], BF, tag="normT")
        nc.vector.tensor_tensor(normT[:, :], avt_ps[:, :], rec[:, :], op=ALU.mult)

        # O^T = W_out_spread.T @ normT + b_out  -> (33,512) with ones row
        o_ps = ps_misc.tile([ATTN, S], F32, tag="misc")
        nc.tensor.matmul(o_ps[:, :], wout_sp[:, :], normT[:, :], start=True, stop=True)
        oT = sb.tile([ATTN + 1, S], BF, tag="oT")
        nc.vector.tensor_scalar_add(oT[0:ATTN, :], o_ps[:, :], bout[:, :])
        nc.vector.memset(oT[ATTN : ATTN + 1, :], 1.0)

        # suppression logits z: (128, 16) free = 4*cc + c
        sup_ps = ps_misc.tile([128, 4 * C], F32, tag="misc")
        for cc in range(4):
            nc.tensor.matmul(
                sup_ps[:, C * cc : C * (cc + 1)],
                oT[:, 128 * cc : 128 * (cc + 1)],
                wsupT_aug[:, :],
                start=True,
                stop=True,
            )
        # 1 + sigmoid(-z) = (2 + e^z) / (1 + e^z); keeps ACT on the exp table
        tb = sb.tile([128, 4 * C], F32, tag="tb")
        nc.scalar.activation(tb[:, :], sup_ps[:, :], ACTF.Exp)
        ab = sb.tile([128, 4 * C], F32, tag="ab")
        nc.vector.tensor_scalar_add(ab[:, :], tb[:, :], 1.0)
        rb = sb.tile([128, 4 * C], F32, tag="rb")
        nc.vector.reciprocal_approx_fast(rb[:, :], ab[:, :])
        ub = sb.tile([128, 4 * C], F32, tag="ub")
        nc.vector.scalar_tensor_tensor(
            ub[:, :], tb[:, :], 2.0, rb[:, :], op0=ALU.add, op1=ALU.mult
        )
        # fw = (1 + sigmoid(-z)) * sharp  (0.5 mean+suppression folded in sharp)
        nc.vector.tensor_tensor(
            fw[:, 16 * b : 16 * (b + 1)],
            ub[:, :],
            sharp[:, 16 * b : 16 * (b + 1)],
            op=ALU.mult,
        )

        # per-batch final gather-contract (3 DVE ops) + store
        gsl = g_all[:, C * EMB * NBJ * b : C * EMB * NBJ * (b + 1)]
        pm = pmat[:, C * EMB * NBJ * b : C * EMB * NBJ * (b + 1)]
        osl = out_all[:, EMB * NBJ * b : EMB * NBJ * (b + 1)]
        nc.vector.tensor_tensor(
            pm.rearrange("p (j e c) -> p j e c", c=C, e=EMB),
            gsl.rearrange("p (j e c) -> p j e c", c=C, e=EMB),
            fw[:, 16 * b : 16 * (b + 1)].rearrange("p (j c) -> p j c", c=C)[
                :, :, None, :
            ].to_broadcast([128, NBJ, EMB, C]),
            op=ALU.mult,
        )
        nc.vector.tensor_reduce(
            osl.rearrange("p (j e) -> p j e", e=EMB),
            pm.rearrange("p (j e c) -> p j e c", c=C, e=EMB),
            axis=mybir.AxisListType.X,
            op=ALU.add,
        )
        nc.vector.tensor_tensor(
            osl.rearrange("p (j e) -> p j e", e=EMB),
            osl.rearrange("p (j e) -> p j e", e=EMB),
            bemb_bc[:, None, :].to_broadcast([128, NBJ, EMB]),
            op=ALU.add,
        )
        nc.sync.dma_start(
            out=io["out"][b, :, :].rearrange("(cc p) e -> p cc e", p=128),
            in_=osl.rearrange("p (cc e) -> p cc e", cc=4),
        )

    ets_l = {b: [None] * 4 for b in range(NB)}
    av_tiles = {}

    def get_av(b):
        if b not in av_tiles:
            av_tiles[b] = (
                ps_av.tile([128, S], F32, tag="avt", name="avt_ps"),
                ps_sum.tile([128, S], F32, tag="sums", name="sums_ps"),
            )
        return av_tiles[b]

    phase_a(0)
    prev = None  # (b, cc) whose AV wave is pending, 1 step behind
    for b in range(NB):
        for cc in range(4):
            ets_l[b][cc] = qk_wave(b, cc)
            if b + 1 < NB:
                A_STAGES[cc](b + 1)
            if prev is not None:
                pb, pcc = prev
                av_wave(pb, pcc, *get_av(pb), ets_l[pb][pcc])
                if pcc == 3:
                    post_batch(pb, *av_tiles[pb])
            prev = (b, cc)
    av_wave(NB - 1, 3, *get_av(NB - 1), ets_l[NB - 1][3])
    post_batch(NB - 1, *av_tiles[NB - 1])


# ======================= host side =======================

def _prep_weights(inp):
    """Pure layout transforms of the parameters (shared by all cores)."""
    f32 = np.float32

    def bf(x):
        return np.ascontiguousarray(np.asarray(x, f32).astype(BF16))

    W_ctx = np.asarray(inp["W_ctx"], f32)
    W_in = np.asarray(inp["W_in"], f32)
    W_out = np.asarray(inp["W_out"], f32)
    W_sup = np.asarray(inp["W_sup"], f32)
    W_emb = np.asarray(inp["W_emb"], f32)
    b_ctx = np.asarray(inp["b_ctx"], f32)
    b_in = np.asarray(inp["b_in"], f32)
    b_out = np.asarray(inp["b_out"], f32)
    b_sup = np.asarray(inp["b_sup"], f32)
    b_emb = np.asarray(inp["b_emb"], f32)
    temp = np.asarray(inp["temperature"], f32)

    w = {}
    w["wctx"] = bf(W_ctx)                                      # (32,64)
    w["bctx_bf"] = bf(b_ctx[:, None])                          # (32,1)
    wq = np.zeros((ATTN, 128), f32)
    wk = np.zeros((ATTN, 128), f32)
    bq = np.zeros((128, 1), f32)
    bk = np.zeros((128, 1), f32)
    for h in range(HEADS):
        wq[:, 32 * h : 32 * h + HD] = W_in[HD * h : HD * (h + 1), :].T
        wk[:, 32 * h : 32 * h + HD] = W_in[ATTN + HD * h : ATTN + HD * (h + 1), :].T
        bq[32 * h : 32 * h + HD, 0] = b_in[HD * h : HD * (h + 1)]
        bk[32 * h : 32 * h + HD, 0] = b_in[ATTN + HD * h : ATTN + HD * (h + 1)]
    w["wq_sp"], w["wk_sp"], w["bq_sp"], w["bk_sp"] = bf(wq), bf(wk), bq, bk
    wv = np.zeros((ATTN + 1, 128), f32)
    for h in range(HEADS):
        wv[0:ATTN, 32 * h : 32 * h + HD] = W_in[2 * ATTN + HD * h : 2 * ATTN + HD * (h + 1), :].T
        wv[ATTN, 32 * h : 32 * h + HD] = b_in[2 * ATTN + HD * h : 2 * ATTN + HD * (h + 1)]
    w["wvT_sp"] = bf(wv)
    wout = np.zeros((128, ATTN), f32)
    for h in range(HEADS):
        wout[32 * h : 32 * h + HD, :] = W_out[:, HD * h : HD * (h + 1)].T
    w["wout_sp"] = bf(wout)
    w["bout"] = np.ascontiguousarray(b_out[:, None])
    w["wsupT_aug"] = bf(np.concatenate([W_sup.T, b_sup[None, :]], 0))  # (33,4)
    w["bemb_bc"] = np.ascontiguousarray(np.broadcast_to(b_emb[None, :], (128, EMB)))
    w["temp"] = np.ascontiguousarray(temp[None, :])
    # W3T[q, e*4+c] = W_emb[e, c*Q+q]  (transpose, c innermost, bf16)
    w["w3T"] = bf(
        W_emb.reshape(EMB, C, Q).transpose(2, 0, 1).reshape(Q, C * EMB)
    )
    return w


def _spec():
    """name -> (shape, mybir dtype) for all per-core DRAM tensors."""
    return {
        "wctx": ((ATTN, EMB), BF), "bctx_bf": ((ATTN, 1), BF),
        "wq_sp": ((ATTN, 128), BF), "wk_sp": ((ATTN, 128), BF),
        "bq_sp": ((128, 1), F32), "bk_sp": ((128, 1), F32),
        "wvT_sp": ((ATTN + 1, 128), BF),
        "wout_sp": ((128, ATTN), BF), "bout": ((ATTN, 1), F32),
        "wsupT_aug": ((ATTN + 1, C), BF),
        "bemb_bc": ((128, EMB), F32), "temp": ((1, HEADS), F32),
        "qidx": ((128, NJ), I32), "rdat": ((128, NJ), I32),
        "w3T": ((Q, C * EMB), BF),
        "ce": ((NB, S, EMB), F32),
    }


def build_bass():
    nc = bacc.Bacc("TRN2", target_bir_lowering=False, debug=False)
    io = {}
    for name, (shape, dt) in _spec().items():
        io[name] = nc.dram_tensor(name, list(shape), dt, kind="ExternalInput").ap()
    io["out"] = nc.dram_tensor("out", [NB, S, EMB], F32, kind="ExternalOutput").ap()
    with tile.TileContext(nc) as tc:
        build_kernel(nc, tc, io)
    nc.compile()
    return nc


def make_in_maps(inputs):
    inp = dict(inputs)
    w = _prep_weights(inp)
    q_idx = np.asarray(inp["q_idx"]).astype(np.int32)
    r_data = np.asarray(inp["r_data"]).astype(np.int32)
    ce = np.asarray(inp["context_embedding"], np.float32)

    in_maps = []
    for k in range(NCORES):
        m = dict(w)
        qs = q_idx[NB * k : NB * (k + 1)]          # (4,512)
        rs = r_data[NB * k : NB * (k + 1)]
        # token-tile layout: [p, j] with j = 4*b + cc, s = 128*cc + p
        m["qidx"] = np.ascontiguousarray(
            qs.reshape(NB, 4, 128).transpose(2, 0, 1).reshape(128, NJ)
        )
        m["rdat"] = np.ascontiguousarray(
            rs.reshape(NB, 4, 128).transpose(2, 0, 1).reshape(128, NJ)
        )
        m["ce"] = np.ascontiguousarray(ce[NB * k : NB * (k + 1)])
        in_maps.append(m)
    return in_maps


_NC_CACHE = {}


def kernel(**inputs) -> np.ndarray:
    if "nc" not in _NC_CACHE:
        _NC_CACHE["nc"] = build_bass()
    nc = _NC_CACHE["nc"]
    in_maps = make_in_maps(inputs)
    res = run_bass_kernel_spmd(nc, in_maps, core_ids=list(range(NCORES)))
    out = np.concatenate([res.results[k]["out"] for k in range(NCORES)], axis=0)
    return out.astype(np.float32)

